# revision 17
# baseline (speedup 1.0000x reference)
"""Bass/Trainium2 kernel for Kimi-style MLA attention (nn_KimiMLAAttention).

Strategy (8 NeuronCores, tensor-parallel over heads):
  - 16 heads -> 2 heads per core. Each core computes q-projection for its 2
    heads, the (replicated) compressed-kv projection + rmsnorm, per-head
    k-embed / v-unembed from the shared latent, causal attention in a
    TRANSPOSED score layout (scores^T[s, l]), and a partial o_proj against
    its 2-head slice of Wo. Host sums the 8 partial outputs.

Performance structure (v2):
  - all matmul operands bf16 (PSUM accumulate fp32); y output bf16.
  - The Tensor engine is in-order, so every slow cross-engine chain is
    staggered: attention e-consumers (colsum / att@v) trail the score
    matmuls by 2 si-units; rmsnorm sum-of-squares matmuls for chunk j are
    emitted inside chunk j+1's projection stream; normalization uses
    reciprocal_approx_fast; o_proj blocks for l-chunk j are emitted in the
    middle of the next chunk's attention stream.
  - PSUM: P0 uses banks pm0..pm6 for the 7 x 128-col projection chunks, and
    bank pm7 holds k_pe rows [0:64] plus the rmsnorm sum-of-squares row at
    partition 64.  Mega phase: ps(3) + po(2) + aux(2) banks, o_proj tiles
    ride the "ps" tag.
"""

from contextlib import ExitStack

import numpy as np

import concourse.bass as bass
import concourse.tile as tile
from concourse import mybir
from concourse.bass import ds, ts
from concourse.bass_utils import run_bass_kernel_spmd

F32 = mybir.dt.float32
RF32 = mybir.dt.float32r
BF = mybir.dt.bfloat16
AF = mybir.ActivationFunctionType


def _patch_tile_tail_drain():
    """walrus's CoreV3 codegen rejects the TileContext tail drain when it
    carries >1 sem waits ("Too many sync wait commands"). Split the waits
    across multiple single-wait drain instructions on the sync engine."""
    if getattr(tile.TileContext, "_tail_drain_patched", False):
        return
    from concourse.vector_clock import ScopedClock

    def _drain_and_barrier(self, tick_clock, wait_clock):
        nc = self.nc
        drain_inst = nc.sync.drain()
        wait_clock.add_sem_waits(
            drain_inst.ins, ScopedClock({None: tick_clock.global_clock})
        )
        inst = drain_inst.ins
        si = inst.sync_info
        if si is not None and si.on_wait is not None and len(si.on_wait) > 1:
            waits = list(si.on_wait)
            upd = list(si.on_update) if si.on_update else []
            inst.sync_info = mybir.SyncInfo(on_wait=waits[:1], on_update=[])
            for i, w in enumerate(waits[1:]):
                extra = nc.sync.drain()
                last = i == len(waits) - 2
                extra.ins.sync_info = mybir.SyncInfo(
                    on_wait=[w], on_update=upd if last else []
                )
        nc.all_engine_barrier()
        assert self.sems is not None
        popped = nc._tile_sem_poison_stack.pop()
        assert popped is self._sem_poison
        nc.clear_and_free_semaphores(list(self.sems.allocated().values()))
        nc.all_engine_barrier()

    tile.TileContext._drain_and_barrier = _drain_and_barrier
    tile.TileContext._tail_drain_patched = True


_patch_tile_tail_drain()


def _split_excess_waits(nc, max_waits=1):
    """walrus's per-instruction sync-wait slots are tiny on this compiler
    build; hoist excess sem waits onto same-engine NoOp carriers placed
    immediately before the instruction (waits fire earlier in the same
    engine stream, so ordering semantics are preserved)."""
    for f in nc.m.functions:
        for bb in f.blocks:
            insts = bb.instructions
            if not any(
                i.sync_info is not None
                and i.sync_info.on_wait
                and len(i.sync_info.on_wait) > max_waits
                for i in insts
            ):
                continue
            out = []
            for inst in insts:
                si = inst.sync_info
                if si is not None and si.on_wait and len(si.on_wait) > max_waits:
                    waits = list(si.on_wait)
                    for w in waits[:-max_waits]:
                        nop = mybir.InstNoOp(
                            name=nc.get_next_instruction_name(), ins=[], outs=[]
                        )
                        nop.engine = inst.engine
                        nop.sync_info = mybir.SyncInfo(on_wait=[w], on_update=[])
                        out.append(nop)
                    inst.sync_info = mybir.SyncInfo(
                        on_wait=waits[-max_waits:],
                        on_update=list(si.on_update) if si.on_update else [],
                    )
                out.append(inst)
            bb.instructions = out


B, L, HID = 1, 2048, 2048
H = 16
NOPE, ROPE, VDIM, LORA = 128, 64, 128, 512
QDIM = NOPE + ROPE
EPS = 1e-5
SCALE = QDIM**-0.5
NCORES = 8
HPC = H // NCORES  # 2 heads per core

LCH = 512  # moving-operand chunk (max fp32 N per matmul / PSUM bank)
NJ = L // LCH  # 4 l-chunks
NK = HID // 128  # 16 contraction tiles for projections
NS = L // 128  # 16 s(key)-tiles
NLAT = LORA // 128  # 4 latent partition tiles
WCOLS = 960  # fused projection weight columns
NXT = 6  # x-tile prefetch depth


def _build_nc():
    nc = bass.Bass()
    xT = nc.dram_tensor("xT", [HID, L], BF, kind="ExternalInput")
    wqkv = nc.dram_tensor("wqkv", [HID, WCOLS], BF, kind="ExternalInput")
    we = nc.dram_tensor("we", [HPC, LORA, NOPE], BF, kind="ExternalInput")
    wu = nc.dram_tensor("wu", [LORA, HPC * VDIM], BF, kind="ExternalInput")
    wo0 = nc.dram_tensor("wo0", [VDIM, HID], BF, kind="ExternalInput")
    wo1 = nc.dram_tensor("wo1", [VDIM, HID], BF, kind="ExternalInput")
    mbig = nc.dram_tensor("mbig", [128, 896], BF, kind="ExternalInput")
    ones_col_d = nc.dram_tensor("ones_col_d", [128, 1], BF, kind="ExternalInput")
    ones_row_d = nc.dram_tensor("ones_row_d", [1, 128], BF, kind="ExternalInput")
    y = nc.dram_tensor("y", [L, HID], BF, kind="ExternalOutput")

    mm = nc.tensor.matmul

    with tile.TileContext(nc) as tc, ExitStack() as ctx:
        persist = ctx.enter_context(tc.tile_pool(name="persist", bufs=1))
        qn = [persist.tile([128, L], BF, name=f"qn{h}", tag=f"qn{h}") for h in range(HPC)]
        qr = persist.tile([128, L], BF, name="qr", tag="qr")
        kpe = persist.tile([128, L], BF, name="kpe", tag="kpe")
        vsb = persist.tile([128, NS * HPC * VDIM], BF, name="vsb", tag="vsb")
        kT = [persist.tile([128, L], BF, name=f"kT{h}", tag=f"kT{h}") for h in range(HPC)]
        outT = [persist.tile([128, L], BF, name=f"outT{h}", tag=f"outT{h}") for h in range(HPC)]
        latT = [persist.tile([128, L], BF, name=f"latT{i}", tag=f"latT{i}") for i in range(NLAT)]
        mask_sb = persist.tile([128, 896], BF, name="mask_sb", tag="mask_sb")
        ones_col = persist.tile([128, 1], BF, name="ones_col", tag="ones_col")
        ones_row = persist.tile([1, 128], BF, name="ones_row", tag="ones_row")
        # all-ones [128,128]: row 64 serves as broadcast lhsT matching the
        # partition-64 base of the rmsnorm stat rows (matmul requires equal
        # lhsT/rhs base partitions)
        ones_sq = persist.tile([128, 128], BF, name="ones_sq", tag="ones_sq")
        eps_col = persist.tile([128, 1], F32, name="eps_col", tag="eps_col")
        # rmsnorm sqrt / inv-sqrt rows live at partition 64 (matching the
        # psum row the sum-of-squares matmul writes), per-j column chunks.
        lnr_all = persist.tile([128, L], F32, name="lnr_all", tag="lnr_all")
        sinv_bf = persist.tile([128, L], BF, name="sinv_bf", tag="sinv_bf")

        nc.sync.dma_start(out=mask_sb, in_=mbig[:, :])
        nc.sync.dma_start(out=ones_col, in_=ones_col_d[:, :])
        nc.sync.dma_start(out=ones_row, in_=ones_row_d[:, :])
        nc.vector.memset(eps_col, EPS)
        nc.vector.memset(ones_sq, 1.0)

        # ---- P0: fused projections + rmsnorm statistics ----
        with (
            tc.tile_pool(name="wq_pool", bufs=1) as wqp,
            tc.tile_pool(name="x_pool", bufs=1) as xp,
            tc.tile_pool(name="sq_pool", bufs=1) as sqp,
            tc.tile_pool(name="ps0", bufs=1, space="PSUM") as pp0,
        ):
            order = [(j, k) for j in range(NJ) for k in range(NK)]
            xtiles = {}

            def issue_xt(idx):
                j, k = order[idx]
                t = xp.tile([128, LCH], BF, name="xt", tag=f"xt{idx % NXT}")
                nc.sync.dma_start(out=t, in_=xT[ts(k, 128), ds(j * LCH, LCH)])
                xtiles[idx] = t

            # interleave first-x and weight DMAs so every k's weight tile
            # lands before the projection k-loop reaches it (queue round-robin)
            w_sb = [wqp.tile([128, WCOLS], BF, name=f"w{k}", tag=f"w{k}") for k in range(NK)]
            for k in range(NK):
                if k < NXT:
                    issue_xt(k)
                nc.sync.dma_start(out=w_sb[k], in_=wqkv[ts(k, 128), :])
            we_sb = []
            for h in range(HPC):
                row = []
                for i in range(NLAT):
                    t = persist.tile([128, NOPE], BF, name=f"we{h}{i}", tag=f"we{h}{i}")
                    nc.sync.dma_start(out=t, in_=we[h, ts(i, 128), :])
                    row.append(t)
                we_sb.append(row)
            wu_sb = []
            for i in range(NLAT):
                t = persist.tile([128, HPC * VDIM], BF, name=f"wu{i}", tag=f"wu{i}")
                nc.sync.dma_start(out=t, in_=wu[ts(i, 128), :])
                wu_sb.append(t)

            # bank pm7: [0:64] = k_pe accumulation, row 64 = sum-of-squares
            pbank7 = pp0.tile([128, LCH], F32, name="pb7", tag="pm7")
            MS7 = [(0, 128), (128, 128), (256, 128), (384, 128), (512, 128), (640, 128), (768, 128)]
            sq_tiles = {}

            def emit_ssq(jj):
                for i in range(NLAT):
                    mm(pbank7[64:65, :], (ones_col), (sq_tiles[jj][i]),
                       start=(i == 0), stop=(i == NLAT - 1))

            def emit_norm_tail(jj):
                # rsqrt(m/LORA + eps) = exp(-0.5 * ln(m/LORA + eps)); both in
                # the natural_log_exp_and_others act table (no table reload)
                nc.scalar.activation(
                    lnr_all[64:65, ds(jj * LCH, LCH)], pbank7[64:65, :],
                    AF.Ln, bias=eps_col[64:65, :], scale=1.0 / LORA)
                nc.scalar.activation(
                    sinv_bf[64:65, ds(jj * LCH, LCH)],
                    lnr_all[64:65, ds(jj * LCH, LCH)], AF.Exp, scale=-0.5)

            for j in range(NJ):
                jc = ds(j * LCH, LCH)
                pss = [pp0.tile([128, LCH], F32, name=f"pm{m}", tag=f"pm{m}") for m in range(7)]
                for k in range(NK):
                    idx = j * NK + k
                    xt = xtiles.pop(idx)
                    for m, (c0, cw) in enumerate(MS7):
                        mm(pss[m], (w_sb[k][:, ds(c0, cw)]), (xt),
                           start=(k == 0), stop=(k == NK - 1))
                    mm(pbank7[0:64, :], (w_sb[k][:, ds(896, 64)]), (xt),
                       start=(k == 0), stop=(k == NK - 1))
                    if idx + NXT < len(order):
                        issue_xt(idx + NXT)
                    if k == 2 and j > 0:
                        emit_ssq(j - 1)
                        emit_norm_tail(j - 1)
                nc.vector.tensor_copy(qn[0][:, jc], pss[0])
                nc.vector.tensor_copy(qn[1][:, jc], pss[1])
                nc.vector.tensor_copy(qr[:, jc], pss[2])
                sql = []
                for i in range(NLAT):
                    nc.vector.tensor_copy(latT[i][:, jc], pss[3 + i])
                    sq = sqp.tile([128, LCH], BF, name="sq", tag=f"sq{i}")
                    nc.scalar.activation(sq, pss[3 + i], AF.Square)
                    sql.append(sq)
                sq_tiles[j] = sql
                nc.vector.tensor_copy(kpe[0:64, jc], pbank7[0:64, :])
                nc.sync.dma_start(out=kpe[64:128, jc], in_=kpe[0:64, jc])
            emit_ssq(NJ - 1)
            emit_norm_tail(NJ - 1)

            wo_sb = []
            for hh, wsrc in enumerate([wo0, wo1]):
                t = persist.tile([128, HID], BF, name=f"wo{hh}", tag=f"wo{hh}")
                nc.sync.dma_start(out=t, in_=wsrc[:, :])
                wo_sb.append(t)

        # ---- mega phase: latent apply + k/v embed + attention + o_proj ----
        with (
            tc.tile_pool(name="epool", bufs=1) as epool,
            tc.tile_pool(name="rows", bufs=1) as rowp,
            tc.tile_pool(name="psm", bufs=1, space="PSUM") as ppm,
        ):
            pending = []  # [delay_units, fn]

            def tick():
                due = []
                for p in pending:
                    p[0] -= 1
                    if p[0] <= 0:
                        due.append(p)
                for p in due:
                    pending.remove(p)
                    p[1]()

            def flush():
                while pending:
                    p = pending.pop(0)
                    p[1]()

            def defer(n, fn):
                pending.append([n, fn])

            ecnt = [0]
            ycnt = [0]
            chunk_psum = {}
            chunk_esum = {}

            def emit_latent_apply(j):
                jc = ds(j * LCH, LCH)
                bc = ppm.tile([128, LCH], F32, name="bc", tag="aux", bufs=2)
                mm(bc, (ones_sq[64:65, :]), (sinv_bf[64:65, jc]),
                   start=True, stop=True)
                for i in range(NLAT):
                    nc.vector.tensor_mul(latT[i][:, jc], latT[i][:, jc], bc)

            def emit_p2_block(j):
                jc = ds(j * LCH, LCH)
                for si in range(4 * j, 4 * j + 4):
                    pv = ppm.tile([128, LCH], F32, name="pv", tag="ps", bufs=3)
                    for i in range(NLAT):
                        mm(pv[:, 0:HPC * VDIM], (latT[i][:, ts(si, 128)]), (wu_sb[i]),
                           start=(i == 0), stop=(i == NLAT - 1))
                    nc.vector.tensor_copy(
                        vsb[:, ds(si * HPC * VDIM, HPC * VDIM)], pv[:, 0:HPC * VDIM])
                for h in range(HPC):
                    pk = ppm.tile([128, LCH], F32, name="pk", tag="po", bufs=3)
                    for i in range(NLAT):
                        mm(pk, (we_sb[h][i]), (latT[i][:, jc]),
                           start=(i == 0), stop=(i == NLAT - 1))
                    nc.vector.tensor_copy(kT[h][:, jc], pk)

            def make_consumer(j, h, si, e, nsi):
                def fn():
                    if si == 0:
                        chunk_psum[(j, h)] = (
                            ppm.tile([128, LCH], F32, name="pcs", tag="aux", bufs=2),
                            ppm.tile([128, LCH], F32, name="po", tag="po", bufs=3),
                        )
                    pcs_t, po_t = chunk_psum[(j, h)]
                    mm(pcs_t[0:1, :], (ones_col), (e),
                       start=(si == 0), stop=(si == nsi - 1))
                    mm(po_t, (vsb[:, ds(si * HPC * VDIM + h * VDIM, VDIM)]), (e),
                       start=(si == 0), stop=(si == nsi - 1))
                return fn

            def make_epilogue(j, h):
                # 1/z = exp(-ln z) on the scalar engine, broadcast via PE,
                # normalize outT (po stays resident until this mul).
                def fn():
                    jc = ds(j * LCH, LCH)
                    pcs_t, po_t = chunk_psum.pop((j, h))
                    rln = rowp.tile([1, LCH], F32, name="rln", tag="rln", bufs=2)
                    nc.scalar.activation(rln, pcs_t[0:1, :], AF.Ln)
                    rrow_bf = rowp.tile([1, LCH], BF, name="rrow_bf", tag="rrow_bf", bufs=2)
                    nc.scalar.activation(rrow_bf, rln, AF.Exp, scale=-1.0)
                    pbc = ppm.tile([128, LCH], F32, name="pbc", tag="aux", bufs=2)
                    mm(pbc, (ones_row), (rrow_bf), start=True, stop=True)
                    bcs = epool.tile([128, LCH], F32, name="bcs", tag="bcs", bufs=2)
                    nc.vector.tensor_copy(bcs, pbc)
                    nc.vector.tensor_mul(outT[h][:, jc], po_t, bcs)
                return fn

            def emit_p4_block(j):
                for i in range(4 * j, 4 * j + 4):
                    for n in range(NJ):
                        py = ppm.tile([128, LCH], F32, name="py", tag="ps", bufs=3)
                        mm(py, (outT[0][:, ts(i, 128)]), (wo_sb[0][:, ds(n * LCH, LCH)]),
                           start=True, stop=False)
                        mm(py, (outT[1][:, ts(i, 128)]), (wo_sb[1][:, ds(n * LCH, LCH)]),
                           start=False, stop=True)
                        ysb = epool.tile([128, LCH], BF, name="ysb", tag="ysb", bufs=4)
                        if ycnt[0] % 2 == 0:
                            nc.scalar.copy(ysb, py)
                        else:
                            nc.vector.tensor_copy(ysb, py)
                        ycnt[0] += 1
                        nc.sync.dma_start(out=y[ts(i, 128), ds(n * LCH, LCH)], in_=ysb)

            for j in range(NJ):
                flush()
                if j == 0:
                    emit_latent_apply(0)
                emit_p2_block(j)
                nsi = 4 * j + 4
                for h in range(HPC):
                    for si in range(nsi):
                        if h == 0 and si == 4 and j > 0:
                            flush()
                            emit_p4_block(j - 1)
                        if h == 1 and si == nsi - 3 and j < NJ - 1:
                            emit_latent_apply(j + 1)
                        jc = ds(j * LCH, LCH)
                        ps_t = ppm.tile([128, LCH], F32, name="ps", tag="ps", bufs=3)
                        mm(ps_t, (kT[h][:, ts(si, 128)]), (qn[h][:, jc]),
                           start=True, stop=False)
                        mm(ps_t, (kpe[ds(h * 64, 64), ts(si, 128)]), (qr[ds(h * 64, 64), jc]),
                           start=False, stop=True)
                        e = epool.tile([128, LCH], BF, name="e", tag=f"e{ecnt[0] % 5}")
                        ecnt[0] += 1
                        nc.scalar.activation(e, ps_t, AF.Exp, scale=SCALE)
                        d = si - 4 * j
                        if d >= 0:
                            nc.gpsimd.tensor_mul(e, e, mask_sb[:, ds(384 - 128 * d, LCH)])
                        tick()
                        defer(3, make_consumer(j, h, si, e, nsi))
                    defer(3, make_epilogue(j, h))
            flush()
            emit_p4_block(NJ - 1)

    _split_excess_waits(nc)
    return nc


_NC_CACHE = None


def _get_nc():
    global _NC_CACHE
    if _NC_CACHE is None:
        _NC_CACHE = _build_nc()
    return _NC_CACHE


def _make_in_maps(x, Wq, Wkv_a, kv_ln_w, W_embed, W_unembed, Wo):
    import ml_dtypes

    BFD = ml_dtypes.bfloat16
    xT = np.ascontiguousarray(np.asarray(x, dtype=np.float32)[0].T).astype(BFD)
    Wq = np.asarray(Wq, dtype=np.float32)
    Wkv_a = np.asarray(Wkv_a, dtype=np.float32)
    kv_ln_w = np.asarray(kv_ln_w, dtype=np.float32)
    W_embed = np.asarray(W_embed, dtype=np.float32)
    W_unembed = np.asarray(W_unembed, dtype=np.float32)
    Wo = np.asarray(Wo, dtype=np.float32)

    Wq3 = Wq.reshape(HID, H, QDIM)
    # diagonal-band mask template: mbig[p, q] = 1 iff (q - 384) >= p
    q_idx = np.arange(896) - 384
    p_idx = np.arange(128)
    mbig = (q_idx[None, :] >= p_idx[:, None]).astype(np.float32)

    in_maps = []
    for c in range(NCORES):
        h0, h1 = HPC * c, HPC * c + 1
        wqkv = np.concatenate(
            [
                Wq3[:, h0, :NOPE],
                Wq3[:, h1, :NOPE],
                Wq3[:, h0, NOPE:],
                Wq3[:, h1, NOPE:],
                Wkv_a,
            ],
            axis=1,
        )
        we = np.ascontiguousarray(W_embed[[h0, h1]] * kv_ln_w[None, :, None])
        wu = np.ascontiguousarray(
            np.concatenate([W_unembed[h0].T, W_unembed[h1].T], axis=1) * kv_ln_w[:, None]
        )
        in_maps.append(
            {
                "xT": xT,
                "wqkv": np.ascontiguousarray(wqkv).astype(BFD),
                "we": we.astype(BFD),
                "wu": wu.astype(BFD),
                "wo0": np.ascontiguousarray(Wo[h0 * VDIM : (h0 + 1) * VDIM]).astype(BFD),
                "wo1": np.ascontiguousarray(Wo[h1 * VDIM : (h1 + 1) * VDIM]).astype(BFD),
                "mbig": mbig.astype(BFD),
                "ones_col_d": np.ones((128, 1), BFD),
                "ones_row_d": np.ones((1, 128), BFD),
            }
        )
    return in_maps


def run(trace=False, tmpdir=None, **inputs):
    """Run the SPMD kernel; returns (full_output, BassKernelResults)."""
    inputs.pop("mask", None)  # causal structure is hardcoded
    nc = _get_nc()
    in_maps = _make_in_maps(**inputs)
    res = run_bass_kernel_spmd(
        nc, in_maps, core_ids=list(range(NCORES)), trace=trace, tmpdir=tmpdir
    )
    y = np.zeros((L, HID), dtype=np.float32)
    for c in range(NCORES):
        y += np.asarray(res.results[c]["y"], dtype=np.float32)
    return y.reshape(B, L, HID), res


def kernel(**inputs):
    y, _ = run(trace=False, **inputs)
    return y


# revision 18
# speedup vs baseline: 1.2063x; 1.2063x over previous
"""Bass/Trainium2 kernel for Kimi-style MLA attention (nn_KimiMLAAttention).

Strategy (8 NeuronCores, tensor-parallel over heads):
  - 16 heads -> 2 heads per core. Each core computes q-projection for its 2
    heads, the (replicated) compressed-kv projection + rmsnorm, per-head
    k-embed / v-unembed from the shared latent, causal attention in a
    TRANSPOSED score layout (scores^T[s, l]), and a partial o_proj against
    its 2-head slice of Wo. Host sums the 8 partial outputs.

Performance structure (v2):
  - all matmul operands bf16 (PSUM accumulate fp32); y output bf16.
  - The Tensor engine is in-order, so every slow cross-engine chain is
    staggered: attention e-consumers (colsum / att@v) trail the score
    matmuls by 2 si-units; rmsnorm sum-of-squares matmuls for chunk j are
    emitted inside chunk j+1's projection stream; normalization uses
    reciprocal_approx_fast; o_proj blocks for l-chunk j are emitted in the
    middle of the next chunk's attention stream.
  - PSUM: P0 uses banks pm0..pm6 for the 7 x 128-col projection chunks, and
    bank pm7 holds k_pe rows [0:64] plus the rmsnorm sum-of-squares row at
    partition 64.  Mega phase: ps(3) + po(2) + aux(2) banks, o_proj tiles
    ride the "ps" tag.
"""

from contextlib import ExitStack

import numpy as np

import concourse.bass as bass
import concourse.tile as tile
from concourse import mybir
from concourse.bass import ds, ts
from concourse.bass_utils import run_bass_kernel_spmd

F32 = mybir.dt.float32
RF32 = mybir.dt.float32r
BF = mybir.dt.bfloat16
AF = mybir.ActivationFunctionType


def _patch_tile_tail_drain():
    """walrus's CoreV3 codegen rejects the TileContext tail drain when it
    carries >1 sem waits ("Too many sync wait commands"). Split the waits
    across multiple single-wait drain instructions on the sync engine."""
    if getattr(tile.TileContext, "_tail_drain_patched", False):
        return
    from concourse.vector_clock import ScopedClock

    def _drain_and_barrier(self, tick_clock, wait_clock):
        nc = self.nc
        drain_inst = nc.sync.drain()
        wait_clock.add_sem_waits(
            drain_inst.ins, ScopedClock({None: tick_clock.global_clock})
        )
        inst = drain_inst.ins
        si = inst.sync_info
        if si is not None and si.on_wait is not None and len(si.on_wait) > 1:
            waits = list(si.on_wait)
            upd = list(si.on_update) if si.on_update else []
            inst.sync_info = mybir.SyncInfo(on_wait=waits[:1], on_update=[])
            for i, w in enumerate(waits[1:]):
                extra = nc.sync.drain()
                last = i == len(waits) - 2
                extra.ins.sync_info = mybir.SyncInfo(
                    on_wait=[w], on_update=upd if last else []
                )
        nc.all_engine_barrier()
        assert self.sems is not None
        popped = nc._tile_sem_poison_stack.pop()
        assert popped is self._sem_poison
        nc.clear_and_free_semaphores(list(self.sems.allocated().values()))
        nc.all_engine_barrier()

    tile.TileContext._drain_and_barrier = _drain_and_barrier
    tile.TileContext._tail_drain_patched = True


_patch_tile_tail_drain()


def _split_excess_waits(nc, max_waits=1):
    """walrus's per-instruction sync-wait slots are tiny on this compiler
    build; hoist excess sem waits onto same-engine NoOp carriers placed
    immediately before the instruction (waits fire earlier in the same
    engine stream, so ordering semantics are preserved)."""
    for f in nc.m.functions:
        for bb in f.blocks:
            insts = bb.instructions
            if not any(
                i.sync_info is not None
                and i.sync_info.on_wait
                and len(i.sync_info.on_wait) > max_waits
                for i in insts
            ):
                continue
            out = []
            for inst in insts:
                si = inst.sync_info
                if si is not None and si.on_wait and len(si.on_wait) > max_waits:
                    waits = list(si.on_wait)
                    for w in waits[:-max_waits]:
                        nop = mybir.InstNoOp(
                            name=nc.get_next_instruction_name(), ins=[], outs=[]
                        )
                        nop.engine = inst.engine
                        nop.sync_info = mybir.SyncInfo(on_wait=[w], on_update=[])
                        out.append(nop)
                    inst.sync_info = mybir.SyncInfo(
                        on_wait=waits[-max_waits:],
                        on_update=list(si.on_update) if si.on_update else [],
                    )
                out.append(inst)
            bb.instructions = out


B, L, HID = 1, 2048, 2048
H = 16
NOPE, ROPE, VDIM, LORA = 128, 64, 128, 512
QDIM = NOPE + ROPE
EPS = 1e-5
SCALE = QDIM**-0.5
NCORES = 8
HPC = H // NCORES  # 2 heads per core

LCH = 512  # moving-operand chunk (max fp32 N per matmul / PSUM bank)
NJ = L // LCH  # 4 l-chunks
NK = HID // 128  # 16 contraction tiles for projections
NS = L // 128  # 16 s(key)-tiles
NLAT = LORA // 128  # 4 latent partition tiles
WCOLS = 960  # fused projection weight columns
NXT = 6  # x-tile prefetch depth


def _build_nc():
    nc = bass.Bass()
    xT = nc.dram_tensor("xT", [HID, L], BF, kind="ExternalInput")
    wqkv = nc.dram_tensor("wqkv", [HID, WCOLS], BF, kind="ExternalInput")
    we = nc.dram_tensor("we", [HPC, LORA, NOPE], BF, kind="ExternalInput")
    wu = nc.dram_tensor("wu", [LORA, HPC * VDIM], BF, kind="ExternalInput")
    wo0 = nc.dram_tensor("wo0", [VDIM, HID], BF, kind="ExternalInput")
    wo1 = nc.dram_tensor("wo1", [VDIM, HID], BF, kind="ExternalInput")
    mbig = nc.dram_tensor("mbig", [128, 896], BF, kind="ExternalInput")
    ones_col_d = nc.dram_tensor("ones_col_d", [128, 1], BF, kind="ExternalInput")
    ones_row_d = nc.dram_tensor("ones_row_d", [1, 128], BF, kind="ExternalInput")
    y = nc.dram_tensor("y", [L, HID], BF, kind="ExternalOutput")

    mm = nc.tensor.matmul

    with tile.TileContext(nc) as tc, ExitStack() as ctx:
        persist = ctx.enter_context(tc.tile_pool(name="persist", bufs=1))
        qn = [persist.tile([128, L], BF, name=f"qn{h}", tag=f"qn{h}") for h in range(HPC)]
        qr = persist.tile([128, L], BF, name="qr", tag="qr")
        kpe = persist.tile([128, L], BF, name="kpe", tag="kpe")
        vsb = persist.tile([128, NS * HPC * VDIM], BF, name="vsb", tag="vsb")
        kT = [persist.tile([128, L], BF, name=f"kT{h}", tag=f"kT{h}") for h in range(HPC)]
        outT = [persist.tile([128, L], BF, name=f"outT{h}", tag=f"outT{h}") for h in range(HPC)]
        latT = [persist.tile([128, L], BF, name=f"latT{i}", tag=f"latT{i}") for i in range(NLAT)]
        mask_sb = persist.tile([128, 896], BF, name="mask_sb", tag="mask_sb")
        ones_col = persist.tile([128, 1], BF, name="ones_col", tag="ones_col")
        ones_row = persist.tile([1, 128], BF, name="ones_row", tag="ones_row")
        # all-ones [128,128]: row 64 serves as broadcast lhsT matching the
        # partition-64 base of the rmsnorm stat rows (matmul requires equal
        # lhsT/rhs base partitions)
        ones_sq = persist.tile([128, 128], BF, name="ones_sq", tag="ones_sq")
        eps_col = persist.tile([128, 1], F32, name="eps_col", tag="eps_col")
        # rmsnorm sqrt / inv-sqrt rows live at partition 64 (matching the
        # psum row the sum-of-squares matmul writes), per-j column chunks.
        lnr_all = persist.tile([128, L], F32, name="lnr_all", tag="lnr_all")
        sinv_bf = persist.tile([128, L], BF, name="sinv_bf", tag="sinv_bf")

        nc.sync.dma_start(out=mask_sb, in_=mbig[:, :])
        nc.sync.dma_start(out=ones_col, in_=ones_col_d[:, :])
        nc.sync.dma_start(out=ones_row, in_=ones_row_d[:, :])
        nc.vector.memset(eps_col, EPS)
        nc.vector.memset(ones_sq, 1.0)

        # ---- P0: fused projections + rmsnorm statistics ----
        with (
            tc.tile_pool(name="wq_pool", bufs=1) as wqp,
            tc.tile_pool(name="x_pool", bufs=1) as xp,
            tc.tile_pool(name="sq_pool", bufs=1) as sqp,
            tc.tile_pool(name="ps0", bufs=1, space="PSUM") as pp0,
        ):
            order = [(j, k) for j in range(NJ) for k in range(NK)]
            xtiles = {}

            def issue_xt(idx):
                j, k = order[idx]
                t = xp.tile([128, LCH], BF, name="xt", tag=f"xt{idx % NXT}")
                nc.sync.dma_start(out=t, in_=xT[ts(k, 128), ds(j * LCH, LCH)])
                xtiles[idx] = t

            # interleave first-x and weight DMAs so every k's weight tile
            # lands before the projection k-loop reaches it (queue round-robin)
            w_sb = [wqp.tile([128, WCOLS], BF, name=f"w{k}", tag=f"w{k}") for k in range(NK)]
            for k in range(NK):
                if k < NXT:
                    issue_xt(k)
                nc.sync.dma_start(out=w_sb[k], in_=wqkv[ts(k, 128), :])
            we_sb = []
            for h in range(HPC):
                row = []
                for i in range(NLAT):
                    t = persist.tile([128, NOPE], BF, name=f"we{h}{i}", tag=f"we{h}{i}")
                    nc.sync.dma_start(out=t, in_=we[h, ts(i, 128), :])
                    row.append(t)
                we_sb.append(row)
            wu_sb = []
            for i in range(NLAT):
                t = persist.tile([128, HPC * VDIM], BF, name=f"wu{i}", tag=f"wu{i}")
                nc.sync.dma_start(out=t, in_=wu[ts(i, 128), :])
                wu_sb.append(t)

            # bank pm7: [0:64] = k_pe accumulation, row 64 = sum-of-squares
            pbank7 = pp0.tile([128, LCH], F32, name="pb7", tag="pm7")
            MS7 = [(0, 128), (128, 128), (256, 128), (384, 128), (512, 128), (640, 128), (768, 128)]
            sq_tiles = {}

            def emit_ssq(jj):
                for i in range(NLAT):
                    mm(pbank7[64:65, :], (ones_col), (sq_tiles[jj][i]),
                       start=(i == 0), stop=(i == NLAT - 1))

            def emit_norm_tail(jj):
                # rsqrt(m/LORA + eps) = exp(-0.5 * ln(m/LORA + eps)); both in
                # the natural_log_exp_and_others act table (no table reload)
                nc.scalar.activation(
                    lnr_all[64:65, ds(jj * LCH, LCH)], pbank7[64:65, :],
                    AF.Ln, bias=eps_col[64:65, :], scale=1.0 / LORA)
                nc.scalar.activation(
                    sinv_bf[64:65, ds(jj * LCH, LCH)],
                    lnr_all[64:65, ds(jj * LCH, LCH)], AF.Exp, scale=-0.5)

            for j in range(NJ):
                jc = ds(j * LCH, LCH)
                pss = [pp0.tile([128, LCH], F32, name=f"pm{m}", tag=f"pm{m}") for m in range(7)]
                for k in range(NK):
                    idx = j * NK + k
                    xt = xtiles.pop(idx)
                    for m, (c0, cw) in enumerate(MS7):
                        mm(pss[m], (w_sb[k][:, ds(c0, cw)]), (xt),
                           start=(k == 0), stop=(k == NK - 1))
                    mm(pbank7[0:64, :], (w_sb[k][:, ds(896, 64)]), (xt),
                       start=(k == 0), stop=(k == NK - 1))
                    if idx + NXT < len(order):
                        issue_xt(idx + NXT)
                    if k == 2 and j > 0:
                        emit_ssq(j - 1)
                        emit_norm_tail(j - 1)
                nc.vector.tensor_copy(qn[0][:, jc], pss[0])
                nc.vector.tensor_copy(qn[1][:, jc], pss[1])
                nc.vector.tensor_copy(qr[:, jc], pss[2])
                sql = []
                for i in range(NLAT):
                    nc.vector.tensor_copy(latT[i][:, jc], pss[3 + i])
                    sq = sqp.tile([128, LCH], BF, name="sq", tag=f"sq{i}")
                    nc.scalar.activation(sq, pss[3 + i], AF.Square)
                    sql.append(sq)
                sq_tiles[j] = sql
                nc.vector.tensor_copy(kpe[0:64, jc], pbank7[0:64, :])
                nc.sync.dma_start(out=kpe[64:128, jc], in_=kpe[0:64, jc])
            emit_ssq(NJ - 1)
            emit_norm_tail(NJ - 1)

            wo_sb = []
            for hh, wsrc in enumerate([wo0, wo1]):
                t = persist.tile([128, HID], BF, name=f"wo{hh}", tag=f"wo{hh}")
                nc.sync.dma_start(out=t, in_=wsrc[:, :])
                wo_sb.append(t)

        # ---- mega phase: latent apply + k/v embed + attention + o_proj ----
        with (
            tc.tile_pool(name="epool", bufs=1) as epool,
            tc.tile_pool(name="rows", bufs=1) as rowp,
            tc.tile_pool(name="psm", bufs=1, space="PSUM") as ppm,
        ):
            pending = []  # [delay_units, fn]

            def tick():
                due = []
                for p in pending:
                    p[0] -= 1
                    if p[0] <= 0:
                        due.append(p)
                for p in due:
                    pending.remove(p)
                    p[1]()

            def flush():
                while pending:
                    p = pending.pop(0)
                    p[1]()

            def defer(n, fn):
                pending.append([n, fn])

            ecnt = [0]
            ycnt = [0]
            chunk_psum = {}
            chunk_esum = {}

            def emit_latent_apply(j):
                jc = ds(j * LCH, LCH)
                bc = ppm.tile([128, LCH], F32, name="bc", tag="aux", bufs=2)
                mm(bc, (ones_sq[64:65, :]), (sinv_bf[64:65, jc]),
                   start=True, stop=True)
                for i in range(NLAT):
                    nc.vector.tensor_mul(latT[i][:, jc], latT[i][:, jc], bc)

            def emit_p2_block(j):
                jc = ds(j * LCH, LCH)
                for si in range(4 * j, 4 * j + 4):
                    pv = ppm.tile([128, LCH], F32, name="pv", tag="ps", bufs=3)
                    for i in range(NLAT):
                        mm(pv[:, 0:HPC * VDIM], (latT[i][:, ts(si, 128)]), (wu_sb[i]),
                           start=(i == 0), stop=(i == NLAT - 1))
                    nc.vector.tensor_copy(
                        vsb[:, ds(si * HPC * VDIM, HPC * VDIM)], pv[:, 0:HPC * VDIM])
                for h in range(HPC):
                    pk = ppm.tile([128, LCH], F32, name="pk", tag="po", bufs=3)
                    for i in range(NLAT):
                        mm(pk, (we_sb[h][i]), (latT[i][:, jc]),
                           start=(i == 0), stop=(i == NLAT - 1))
                    nc.vector.tensor_copy(kT[h][:, jc], pk)

            def make_consumer(j, h, si, e, nsi):
                def fn():
                    if si == 0:
                        chunk_psum[(j, h)] = ppm.tile(
                            [128, LCH], F32, name="po", tag="po", bufs=3)
                    po_t = chunk_psum[(j, h)]
                    mm(po_t, (vsb[:, ds(si * HPC * VDIM + h * VDIM, VDIM)]), (e),
                       start=(si == 0), stop=(si == nsi - 1))
                return fn

            def make_epilogue(j, h):
                # E1: partition-sum of esum via ones matmul; E2 (2 units
                # later): 1/z = exp(-ln z), broadcast via PE, normalize outT.
                def fn():
                    es = chunk_esum.pop((j, h))
                    pcs_t = ppm.tile([128, LCH], F32, name="pcs", tag="aux", bufs=2)
                    mm(pcs_t[0:1, :], (ones_col), (es), start=True, stop=True)

                    def fn2():
                        jc = ds(j * LCH, LCH)
                        po_t = chunk_psum.pop((j, h))
                        rln = rowp.tile([1, LCH], F32, name="rln", tag="rln", bufs=2)
                        nc.scalar.activation(rln, pcs_t[0:1, :], AF.Ln)
                        rrow_bf = rowp.tile([1, LCH], BF, name="rrow_bf", tag="rrow_bf", bufs=2)
                        nc.scalar.activation(rrow_bf, rln, AF.Exp, scale=-1.0)
                        pbc = ppm.tile([128, LCH], F32, name="pbc", tag="aux", bufs=2)
                        mm(pbc, (ones_row), (rrow_bf), start=True, stop=True)
                        bcs = epool.tile([128, LCH], F32, name="bcs", tag="bcs", bufs=2)
                        nc.vector.tensor_copy(bcs, pbc)
                        nc.vector.tensor_mul(outT[h][:, jc], po_t, bcs)

                    defer(2, fn2)
                return fn

            def emit_p4_block(j):
                for i in range(4 * j, 4 * j + 4):
                    for n in range(NJ):
                        py = ppm.tile([128, LCH], F32, name="py", tag="ps", bufs=3)
                        mm(py, (outT[0][:, ts(i, 128)]), (wo_sb[0][:, ds(n * LCH, LCH)]),
                           start=True, stop=False)
                        mm(py, (outT[1][:, ts(i, 128)]), (wo_sb[1][:, ds(n * LCH, LCH)]),
                           start=False, stop=True)
                        ysb = epool.tile([128, LCH], BF, name="ysb", tag="ysb", bufs=4)
                        if ycnt[0] % 2 == 0:
                            nc.scalar.copy(ysb, py)
                        else:
                            nc.vector.tensor_copy(ysb, py)
                        ycnt[0] += 1
                        nc.sync.dma_start(out=y[ts(i, 128), ds(n * LCH, LCH)], in_=ysb)

            for j in range(NJ):
                flush()
                if j == 0:
                    emit_latent_apply(0)
                emit_p2_block(j)
                nsi = 4 * j + 4
                for h in range(HPC):
                    for si in range(nsi):
                        if h == 0 and si == 4 and j > 0:
                            flush()
                            emit_p4_block(j - 1)
                        if h == 1 and si == nsi - 3 and j < NJ - 1:
                            emit_latent_apply(j + 1)
                        jc = ds(j * LCH, LCH)
                        ps_t = ppm.tile([128, LCH], F32, name="ps", tag="ps", bufs=3)
                        mm(ps_t, (kT[h][:, ts(si, 128)]), (qn[h][:, jc]),
                           start=True, stop=False)
                        mm(ps_t, (kpe[ds(h * 64, 64), ts(si, 128)]), (qr[ds(h * 64, 64), jc]),
                           start=False, stop=True)
                        e = epool.tile([128, LCH], BF, name="e", tag=f"e{ecnt[0] % 5}")
                        ecnt[0] += 1
                        nc.scalar.activation(e, ps_t, AF.Exp, scale=SCALE)
                        d = si - 4 * j
                        if d >= 0:
                            nc.gpsimd.tensor_mul(e, e, mask_sb[:, ds(384 - 128 * d, LCH)])
                        if si == 0:
                            es = epool.tile([128, LCH], BF, name="esum",
                                            tag=f"esum{(2 * j + h) % 2}")
                            chunk_esum[(j, h)] = es
                            nc.vector.tensor_copy(es, e)
                        else:
                            es = chunk_esum[(j, h)]
                            nc.vector.tensor_add(es, es, e)
                        tick()
                        defer(3, make_consumer(j, h, si, e, nsi))
                    defer(3, make_epilogue(j, h))
            flush()
            emit_p4_block(NJ - 1)

    _split_excess_waits(nc)
    return nc


_NC_CACHE = None


def _get_nc():
    global _NC_CACHE
    if _NC_CACHE is None:
        _NC_CACHE = _build_nc()
    return _NC_CACHE


def _make_in_maps(x, Wq, Wkv_a, kv_ln_w, W_embed, W_unembed, Wo):
    import ml_dtypes

    BFD = ml_dtypes.bfloat16
    xT = np.ascontiguousarray(np.asarray(x, dtype=np.float32)[0].T).astype(BFD)
    Wq = np.asarray(Wq, dtype=np.float32)
    Wkv_a = np.asarray(Wkv_a, dtype=np.float32)
    kv_ln_w = np.asarray(kv_ln_w, dtype=np.float32)
    W_embed = np.asarray(W_embed, dtype=np.float32)
    W_unembed = np.asarray(W_unembed, dtype=np.float32)
    Wo = np.asarray(Wo, dtype=np.float32)

    Wq3 = Wq.reshape(HID, H, QDIM)
    # diagonal-band mask template: mbig[p, q] = 1 iff (q - 384) >= p
    q_idx = np.arange(896) - 384
    p_idx = np.arange(128)
    mbig = (q_idx[None, :] >= p_idx[:, None]).astype(np.float32)

    in_maps = []
    for c in range(NCORES):
        h0, h1 = HPC * c, HPC * c + 1
        wqkv = np.concatenate(
            [
                Wq3[:, h0, :NOPE],
                Wq3[:, h1, :NOPE],
                Wq3[:, h0, NOPE:],
                Wq3[:, h1, NOPE:],
                Wkv_a,
            ],
            axis=1,
        )
        we = np.ascontiguousarray(W_embed[[h0, h1]] * kv_ln_w[None, :, None])
        wu = np.ascontiguousarray(
            np.concatenate([W_unembed[h0].T, W_unembed[h1].T], axis=1) * kv_ln_w[:, None]
        )
        in_maps.append(
            {
                "xT": xT,
                "wqkv": np.ascontiguousarray(wqkv).astype(BFD),
                "we": we.astype(BFD),
                "wu": wu.astype(BFD),
                "wo0": np.ascontiguousarray(Wo[h0 * VDIM : (h0 + 1) * VDIM]).astype(BFD),
                "wo1": np.ascontiguousarray(Wo[h1 * VDIM : (h1 + 1) * VDIM]).astype(BFD),
                "mbig": mbig.astype(BFD),
                "ones_col_d": np.ones((128, 1), BFD),
                "ones_row_d": np.ones((1, 128), BFD),
            }
        )
    return in_maps


def run(trace=False, tmpdir=None, **inputs):
    """Run the SPMD kernel; returns (full_output, BassKernelResults)."""
    inputs.pop("mask", None)  # causal structure is hardcoded
    nc = _get_nc()
    in_maps = _make_in_maps(**inputs)
    res = run_bass_kernel_spmd(
        nc, in_maps, core_ids=list(range(NCORES)), trace=trace, tmpdir=tmpdir
    )
    y = np.zeros((L, HID), dtype=np.float32)
    for c in range(NCORES):
        y += np.asarray(res.results[c]["y"], dtype=np.float32)
    return y.reshape(B, L, HID), res


def kernel(**inputs):
    y, _ = run(trace=False, **inputs)
    return y


# revision 19
# speedup vs baseline: 1.2226x; 1.0136x over previous
"""Bass/Trainium2 kernel for Kimi-style MLA attention (nn_KimiMLAAttention).

Strategy (8 NeuronCores, tensor-parallel over heads):
  - 16 heads -> 2 heads per core. Each core computes q-projection for its 2
    heads, the (replicated) compressed-kv projection + rmsnorm, per-head
    k-embed / v-unembed from the shared latent, causal attention in a
    TRANSPOSED score layout (scores^T[s, l]), and a partial o_proj against
    its 2-head slice of Wo. Host sums the 8 partial outputs.

Performance structure (v2):
  - all matmul operands bf16 (PSUM accumulate fp32); y output bf16.
  - The Tensor engine is in-order, so every slow cross-engine chain is
    staggered: attention e-consumers (colsum / att@v) trail the score
    matmuls by 2 si-units; rmsnorm sum-of-squares matmuls for chunk j are
    emitted inside chunk j+1's projection stream; normalization uses
    reciprocal_approx_fast; o_proj blocks for l-chunk j are emitted in the
    middle of the next chunk's attention stream.
  - PSUM: P0 uses banks pm0..pm6 for the 7 x 128-col projection chunks, and
    bank pm7 holds k_pe rows [0:64] plus the rmsnorm sum-of-squares row at
    partition 64.  Mega phase: ps(3) + po(2) + aux(2) banks, o_proj tiles
    ride the "ps" tag.
"""

from contextlib import ExitStack

import numpy as np

import concourse.bass as bass
import concourse.tile as tile
from concourse import mybir
from concourse.bass import ds, ts
from concourse.bass_utils import run_bass_kernel_spmd

F32 = mybir.dt.float32
RF32 = mybir.dt.float32r
BF = mybir.dt.bfloat16
AF = mybir.ActivationFunctionType


def _patch_tile_tail_drain():
    """walrus's CoreV3 codegen rejects the TileContext tail drain when it
    carries >1 sem waits ("Too many sync wait commands"). Split the waits
    across multiple single-wait drain instructions on the sync engine."""
    if getattr(tile.TileContext, "_tail_drain_patched", False):
        return
    from concourse.vector_clock import ScopedClock

    def _drain_and_barrier(self, tick_clock, wait_clock):
        nc = self.nc
        drain_inst = nc.sync.drain()
        wait_clock.add_sem_waits(
            drain_inst.ins, ScopedClock({None: tick_clock.global_clock})
        )
        inst = drain_inst.ins
        si = inst.sync_info
        if si is not None and si.on_wait is not None and len(si.on_wait) > 1:
            waits = list(si.on_wait)
            upd = list(si.on_update) if si.on_update else []
            inst.sync_info = mybir.SyncInfo(on_wait=waits[:1], on_update=[])
            for i, w in enumerate(waits[1:]):
                extra = nc.sync.drain()
                last = i == len(waits) - 2
                extra.ins.sync_info = mybir.SyncInfo(
                    on_wait=[w], on_update=upd if last else []
                )
        nc.all_engine_barrier()
        assert self.sems is not None
        popped = nc._tile_sem_poison_stack.pop()
        assert popped is self._sem_poison
        nc.clear_and_free_semaphores(list(self.sems.allocated().values()))
        nc.all_engine_barrier()

    tile.TileContext._drain_and_barrier = _drain_and_barrier
    tile.TileContext._tail_drain_patched = True


_patch_tile_tail_drain()


def _split_excess_waits(nc, max_waits=1):
    """walrus's per-instruction sync-wait slots are tiny on this compiler
    build; hoist excess sem waits onto same-engine NoOp carriers placed
    immediately before the instruction (waits fire earlier in the same
    engine stream, so ordering semantics are preserved)."""
    for f in nc.m.functions:
        for bb in f.blocks:
            insts = bb.instructions
            if not any(
                i.sync_info is not None
                and i.sync_info.on_wait
                and len(i.sync_info.on_wait) > max_waits
                for i in insts
            ):
                continue
            out = []
            for inst in insts:
                si = inst.sync_info
                if si is not None and si.on_wait and len(si.on_wait) > max_waits:
                    waits = list(si.on_wait)
                    for w in waits[:-max_waits]:
                        nop = mybir.InstNoOp(
                            name=nc.get_next_instruction_name(), ins=[], outs=[]
                        )
                        nop.engine = inst.engine
                        nop.sync_info = mybir.SyncInfo(on_wait=[w], on_update=[])
                        out.append(nop)
                    inst.sync_info = mybir.SyncInfo(
                        on_wait=waits[-max_waits:],
                        on_update=list(si.on_update) if si.on_update else [],
                    )
                out.append(inst)
            bb.instructions = out


B, L, HID = 1, 2048, 2048
H = 16
NOPE, ROPE, VDIM, LORA = 128, 64, 128, 512
QDIM = NOPE + ROPE
EPS = 1e-5
SCALE = QDIM**-0.5
NCORES = 8
HPC = H // NCORES  # 2 heads per core

LCH = 512  # moving-operand chunk (max fp32 N per matmul / PSUM bank)
NJ = L // LCH  # 4 l-chunks
NK = HID // 128  # 16 contraction tiles for projections
NS = L // 128  # 16 s(key)-tiles
NLAT = LORA // 128  # 4 latent partition tiles
WCOLS = 960  # fused projection weight columns
NXT = 6  # x-tile prefetch depth


def _build_nc():
    nc = bass.Bass()
    xT = nc.dram_tensor("xT", [HID, L], BF, kind="ExternalInput")
    wqkv = nc.dram_tensor("wqkv", [HID, WCOLS], BF, kind="ExternalInput")
    we = nc.dram_tensor("we", [HPC, LORA, NOPE], BF, kind="ExternalInput")
    wu = nc.dram_tensor("wu", [LORA, HPC * VDIM], BF, kind="ExternalInput")
    wo0 = nc.dram_tensor("wo0", [VDIM, HID], BF, kind="ExternalInput")
    wo1 = nc.dram_tensor("wo1", [VDIM, HID], BF, kind="ExternalInput")
    mbig = nc.dram_tensor("mbig", [128, 896], BF, kind="ExternalInput")
    ones_col_d = nc.dram_tensor("ones_col_d", [128, 1], BF, kind="ExternalInput")
    ones_row_d = nc.dram_tensor("ones_row_d", [1, 128], BF, kind="ExternalInput")
    y = nc.dram_tensor("y", [L, HID], BF, kind="ExternalOutput")

    mm = nc.tensor.matmul

    with tile.TileContext(nc) as tc, ExitStack() as ctx:
        persist = ctx.enter_context(tc.tile_pool(name="persist", bufs=1))
        qn = [persist.tile([128, L], BF, name=f"qn{h}", tag=f"qn{h}") for h in range(HPC)]
        qr = persist.tile([128, L], BF, name="qr", tag="qr")
        kpe = persist.tile([128, L], BF, name="kpe", tag="kpe")
        vsb = persist.tile([128, NS * HPC * VDIM], BF, name="vsb", tag="vsb")
        kT = [persist.tile([128, L], BF, name=f"kT{h}", tag=f"kT{h}") for h in range(HPC)]
        outT = [persist.tile([128, L], BF, name=f"outT{h}", tag=f"outT{h}") for h in range(HPC)]
        latT = [persist.tile([128, L], BF, name=f"latT{i}", tag=f"latT{i}") for i in range(NLAT)]
        mask_sb = persist.tile([128, 896], BF, name="mask_sb", tag="mask_sb")
        ones_col = persist.tile([128, 1], BF, name="ones_col", tag="ones_col")
        ones_row = persist.tile([1, 128], BF, name="ones_row", tag="ones_row")
        # all-ones [128,128]: row 64 serves as broadcast lhsT matching the
        # partition-64 base of the rmsnorm stat rows (matmul requires equal
        # lhsT/rhs base partitions)
        ones_sq = persist.tile([128, 128], BF, name="ones_sq", tag="ones_sq")
        eps_col = persist.tile([128, 1], F32, name="eps_col", tag="eps_col")
        # rmsnorm sqrt / inv-sqrt rows live at partition 64 (matching the
        # psum row the sum-of-squares matmul writes), per-j column chunks.
        lnr_all = persist.tile([128, L], F32, name="lnr_all", tag="lnr_all")
        sinv_bf = persist.tile([128, L], BF, name="sinv_bf", tag="sinv_bf")

        nc.sync.dma_start(out=mask_sb, in_=mbig[:, :])
        nc.sync.dma_start(out=ones_col, in_=ones_col_d[:, :])
        nc.sync.dma_start(out=ones_row, in_=ones_row_d[:, :])
        nc.vector.memset(eps_col, EPS)
        nc.vector.memset(ones_sq, 1.0)

        # ---- P0: fused projections + rmsnorm statistics ----
        with (
            tc.tile_pool(name="wq_pool", bufs=1) as wqp,
            tc.tile_pool(name="x_pool", bufs=1) as xp,
            tc.tile_pool(name="sq_pool", bufs=1) as sqp,
            tc.tile_pool(name="ps0", bufs=1, space="PSUM") as pp0,
        ):
            order = [(j, k) for j in range(NJ) for k in range(NK)]
            xtiles = {}

            def issue_xt(idx):
                j, k = order[idx]
                t = xp.tile([128, LCH], BF, name="xt", tag=f"xt{idx % NXT}")
                nc.sync.dma_start(out=t, in_=xT[ts(k, 128), ds(j * LCH, LCH)])
                xtiles[idx] = t

            # interleave first-x and weight DMAs so every k's weight tile
            # lands before the projection k-loop reaches it (queue round-robin)
            w_sb = [wqp.tile([128, WCOLS], BF, name=f"w{k}", tag=f"w{k}") for k in range(NK)]
            for k in range(NK):
                if k < NXT:
                    issue_xt(k)
                nc.sync.dma_start(out=w_sb[k], in_=wqkv[ts(k, 128), :])
            we_sb = []
            for h in range(HPC):
                row = []
                for i in range(NLAT):
                    t = persist.tile([128, NOPE], BF, name=f"we{h}{i}", tag=f"we{h}{i}")
                    nc.sync.dma_start(out=t, in_=we[h, ts(i, 128), :])
                    row.append(t)
                we_sb.append(row)
            wu_sb = []
            for i in range(NLAT):
                t = persist.tile([128, HPC * VDIM], BF, name=f"wu{i}", tag=f"wu{i}")
                nc.sync.dma_start(out=t, in_=wu[ts(i, 128), :])
                wu_sb.append(t)

            # bank pm7: [0:64] = k_pe accumulation, row 64 = sum-of-squares
            pbank7 = pp0.tile([128, LCH], F32, name="pb7", tag="pm7")
            MS7 = [(0, 128), (128, 128), (256, 128), (384, 128), (512, 128), (640, 128), (768, 128)]
            sq_tiles = {}

            def emit_ssq(jj):
                for i in range(NLAT):
                    mm(pbank7[64:65, :], (ones_col), (sq_tiles[jj][i]),
                       start=(i == 0), stop=(i == NLAT - 1))

            def emit_norm_tail(jj):
                # rsqrt(m/LORA + eps) = exp(-0.5 * ln(m/LORA + eps)); both in
                # the natural_log_exp_and_others act table (no table reload)
                nc.scalar.activation(
                    lnr_all[64:65, ds(jj * LCH, LCH)], pbank7[64:65, :],
                    AF.Ln, bias=eps_col[64:65, :], scale=1.0 / LORA)
                nc.scalar.activation(
                    sinv_bf[64:65, ds(jj * LCH, LCH)],
                    lnr_all[64:65, ds(jj * LCH, LCH)], AF.Exp, scale=-0.5)

            for j in range(NJ):
                jc = ds(j * LCH, LCH)
                pss = [pp0.tile([128, LCH], F32, name=f"pm{m}", tag=f"pm{m}") for m in range(7)]
                for k in range(NK):
                    idx = j * NK + k
                    xt = xtiles.pop(idx)
                    for m, (c0, cw) in enumerate(MS7):
                        mm(pss[m], (w_sb[k][:, ds(c0, cw)]), (xt),
                           start=(k == 0), stop=(k == NK - 1))
                    mm(pbank7[0:64, :], (w_sb[k][:, ds(896, 64)]), (xt),
                       start=(k == 0), stop=(k == NK - 1))
                    if idx + NXT < len(order):
                        issue_xt(idx + NXT)
                    if k == 2 and j > 0:
                        emit_ssq(j - 1)
                        emit_norm_tail(j - 1)
                nc.vector.tensor_copy(qn[0][:, jc], pss[0])
                nc.vector.tensor_copy(qn[1][:, jc], pss[1])
                nc.vector.tensor_copy(qr[:, jc], pss[2])
                sql = []
                for i in range(NLAT):
                    nc.vector.tensor_copy(latT[i][:, jc], pss[3 + i])
                    sq = sqp.tile([128, LCH], BF, name="sq", tag=f"sq{i}")
                    nc.scalar.activation(sq, pss[3 + i], AF.Square)
                    sql.append(sq)
                sq_tiles[j] = sql
                nc.vector.tensor_copy(kpe[0:64, jc], pbank7[0:64, :])
                nc.sync.dma_start(out=kpe[64:128, jc], in_=kpe[0:64, jc])
            emit_ssq(NJ - 1)
            emit_norm_tail(NJ - 1)
            # apply rmsnorm to the j=0 latent chunk here: pm0's bank is free
            # after its copy, and the mega phase can then start P2 immediately
            bc0 = pp0.tile([128, LCH], F32, name="bc0", tag="pm0")
            mm(bc0, (ones_sq[64:65, :]), (sinv_bf[64:65, 0:LCH]),
               start=True, stop=True)
            for i in range(NLAT):
                nc.vector.tensor_mul(latT[i][:, 0:LCH], latT[i][:, 0:LCH], bc0)

            wo_sb = []
            for hh, wsrc in enumerate([wo0, wo1]):
                t = persist.tile([128, HID], BF, name=f"wo{hh}", tag=f"wo{hh}")
                nc.sync.dma_start(out=t, in_=wsrc[:, :])
                wo_sb.append(t)

        # ---- mega phase: latent apply + k/v embed + attention + o_proj ----
        with (
            tc.tile_pool(name="epool", bufs=1) as epool,
            tc.tile_pool(name="rows", bufs=1) as rowp,
            tc.tile_pool(name="psm", bufs=1, space="PSUM") as ppm,
        ):
            pending = []  # [delay_units, fn]

            def tick():
                due = []
                for p in pending:
                    p[0] -= 1
                    if p[0] <= 0:
                        due.append(p)
                for p in due:
                    pending.remove(p)
                    p[1]()

            def flush():
                while pending:
                    p = pending.pop(0)
                    p[1]()

            def defer(n, fn):
                pending.append([n, fn])

            ecnt = [0]
            ycnt = [0]
            chunk_psum = {}
            chunk_esum = {}

            def emit_latent_apply(j):
                jc = ds(j * LCH, LCH)
                bc = ppm.tile([128, LCH], F32, name="bc", tag="aux", bufs=2)
                mm(bc, (ones_sq[64:65, :]), (sinv_bf[64:65, jc]),
                   start=True, stop=True)
                for i in range(NLAT):
                    nc.vector.tensor_mul(latT[i][:, jc], latT[i][:, jc], bc)

            def emit_p2_block(j):
                jc = ds(j * LCH, LCH)
                for si in range(4 * j, 4 * j + 4):
                    pv = ppm.tile([128, LCH], F32, name="pv", tag="ps", bufs=3)
                    for i in range(NLAT):
                        mm(pv[:, 0:HPC * VDIM], (latT[i][:, ts(si, 128)]), (wu_sb[i]),
                           start=(i == 0), stop=(i == NLAT - 1))
                    nc.vector.tensor_copy(
                        vsb[:, ds(si * HPC * VDIM, HPC * VDIM)], pv[:, 0:HPC * VDIM])
                for h in range(HPC):
                    pk = ppm.tile([128, LCH], F32, name="pk", tag="po", bufs=3)
                    for i in range(NLAT):
                        mm(pk, (we_sb[h][i]), (latT[i][:, jc]),
                           start=(i == 0), stop=(i == NLAT - 1))
                    nc.vector.tensor_copy(kT[h][:, jc], pk)

            def make_consumer(j, h, si, e, nsi):
                def fn():
                    if si == 0:
                        chunk_psum[(j, h)] = ppm.tile(
                            [128, LCH], F32, name="po", tag="po", bufs=3)
                    po_t = chunk_psum[(j, h)]
                    mm(po_t, (vsb[:, ds(si * HPC * VDIM + h * VDIM, VDIM)]), (e),
                       start=(si == 0), stop=(si == nsi - 1))
                return fn

            def make_epilogue(j, h):
                # E1: partition-sum of esum via ones matmul; E2 (2 units
                # later): 1/z = exp(-ln z), broadcast via PE, normalize outT.
                def fn():
                    es = chunk_esum.pop((j, h))
                    pcs_t = ppm.tile([128, LCH], F32, name="pcs", tag="aux", bufs=2)
                    mm(pcs_t[0:1, :], (ones_col), (es), start=True, stop=True)

                    def fn2():
                        jc = ds(j * LCH, LCH)
                        po_t = chunk_psum.pop((j, h))
                        rln = rowp.tile([1, LCH], F32, name="rln", tag="rln", bufs=2)
                        nc.scalar.activation(rln, pcs_t[0:1, :], AF.Ln)
                        rrow_bf = rowp.tile([1, LCH], BF, name="rrow_bf", tag="rrow_bf", bufs=2)
                        nc.scalar.activation(rrow_bf, rln, AF.Exp, scale=-1.0)
                        pbc = ppm.tile([128, LCH], F32, name="pbc", tag="aux", bufs=2)
                        mm(pbc, (ones_row), (rrow_bf), start=True, stop=True)
                        bcs = epool.tile([128, LCH], F32, name="bcs", tag="bcs", bufs=2)
                        nc.vector.tensor_copy(bcs, pbc)
                        nc.vector.tensor_mul(outT[h][:, jc], po_t, bcs)

                    defer(2, fn2)
                return fn

            def emit_p4_block(j):
                for i in range(4 * j, 4 * j + 4):
                    for n in range(NJ):
                        py = ppm.tile([128, LCH], F32, name="py", tag="ps", bufs=3)
                        mm(py, (outT[0][:, ts(i, 128)]), (wo_sb[0][:, ds(n * LCH, LCH)]),
                           start=True, stop=False)
                        mm(py, (outT[1][:, ts(i, 128)]), (wo_sb[1][:, ds(n * LCH, LCH)]),
                           start=False, stop=True)
                        ysb = epool.tile([128, LCH], BF, name="ysb", tag="ysb", bufs=4)
                        if ycnt[0] % 2 == 0:
                            nc.scalar.copy(ysb, py)
                        else:
                            nc.vector.tensor_copy(ysb, py)
                        ycnt[0] += 1
                        nc.sync.dma_start(out=y[ts(i, 128), ds(n * LCH, LCH)], in_=ysb)

            for j in range(NJ):
                flush()
                emit_p2_block(j)
                nsi = 4 * j + 4
                for h in range(HPC):
                    for si in range(nsi):
                        if h == 0 and si == 4 and j > 0:
                            # no flush: the o_proj block only allocates
                            # exp-gated "ps" tiles, so pending consumers can
                            # stay queued (they fire at the next tick)
                            emit_p4_block(j - 1)
                        if h == 1 and si == nsi - 3 and j < NJ - 1:
                            emit_latent_apply(j + 1)
                        jc = ds(j * LCH, LCH)
                        ps_t = ppm.tile([128, LCH], F32, name="ps", tag="ps", bufs=3)
                        mm(ps_t, (kT[h][:, ts(si, 128)]), (qn[h][:, jc]),
                           start=True, stop=False)
                        mm(ps_t, (kpe[ds(h * 64, 64), ts(si, 128)]), (qr[ds(h * 64, 64), jc]),
                           start=False, stop=True)
                        e = epool.tile([128, LCH], BF, name="e", tag=f"e{ecnt[0] % 5}")
                        ecnt[0] += 1
                        nc.scalar.activation(e, ps_t, AF.Exp, scale=SCALE)
                        d = si - 4 * j
                        if d >= 0:
                            nc.gpsimd.tensor_mul(e, e, mask_sb[:, ds(384 - 128 * d, LCH)])
                        if si == 0:
                            es = epool.tile([128, LCH], BF, name="esum",
                                            tag=f"esum{(2 * j + h) % 2}")
                            chunk_esum[(j, h)] = es
                            nc.vector.tensor_copy(es, e)
                        else:
                            es = chunk_esum[(j, h)]
                            nc.vector.tensor_add(es, es, e)
                        tick()
                        defer(3, make_consumer(j, h, si, e, nsi))
                    defer(3, make_epilogue(j, h))
            flush()
            emit_p4_block(NJ - 1)

    _split_excess_waits(nc)
    return nc


_NC_CACHE = None


def _get_nc():
    global _NC_CACHE
    if _NC_CACHE is None:
        _NC_CACHE = _build_nc()
    return _NC_CACHE


def _make_in_maps(x, Wq, Wkv_a, kv_ln_w, W_embed, W_unembed, Wo):
    import ml_dtypes

    BFD = ml_dtypes.bfloat16
    xT = np.ascontiguousarray(np.asarray(x, dtype=np.float32)[0].T).astype(BFD)
    Wq = np.asarray(Wq, dtype=np.float32)
    Wkv_a = np.asarray(Wkv_a, dtype=np.float32)
    kv_ln_w = np.asarray(kv_ln_w, dtype=np.float32)
    W_embed = np.asarray(W_embed, dtype=np.float32)
    W_unembed = np.asarray(W_unembed, dtype=np.float32)
    Wo = np.asarray(Wo, dtype=np.float32)

    Wq3 = Wq.reshape(HID, H, QDIM)
    # diagonal-band mask template: mbig[p, q] = 1 iff (q - 384) >= p
    q_idx = np.arange(896) - 384
    p_idx = np.arange(128)
    mbig = (q_idx[None, :] >= p_idx[:, None]).astype(np.float32)

    in_maps = []
    for c in range(NCORES):
        h0, h1 = HPC * c, HPC * c + 1
        wqkv = np.concatenate(
            [
                Wq3[:, h0, :NOPE],
                Wq3[:, h1, :NOPE],
                Wq3[:, h0, NOPE:],
                Wq3[:, h1, NOPE:],
                Wkv_a,
            ],
            axis=1,
        )
        we = np.ascontiguousarray(W_embed[[h0, h1]] * kv_ln_w[None, :, None])
        wu = np.ascontiguousarray(
            np.concatenate([W_unembed[h0].T, W_unembed[h1].T], axis=1) * kv_ln_w[:, None]
        )
        in_maps.append(
            {
                "xT": xT,
                "wqkv": np.ascontiguousarray(wqkv).astype(BFD),
                "we": we.astype(BFD),
                "wu": wu.astype(BFD),
                "wo0": np.ascontiguousarray(Wo[h0 * VDIM : (h0 + 1) * VDIM]).astype(BFD),
                "wo1": np.ascontiguousarray(Wo[h1 * VDIM : (h1 + 1) * VDIM]).astype(BFD),
                "mbig": mbig.astype(BFD),
                "ones_col_d": np.ones((128, 1), BFD),
                "ones_row_d": np.ones((1, 128), BFD),
            }
        )
    return in_maps


def run(trace=False, tmpdir=None, **inputs):
    """Run the SPMD kernel; returns (full_output, BassKernelResults)."""
    inputs.pop("mask", None)  # causal structure is hardcoded
    nc = _get_nc()
    in_maps = _make_in_maps(**inputs)
    res = run_bass_kernel_spmd(
        nc, in_maps, core_ids=list(range(NCORES)), trace=trace, tmpdir=tmpdir
    )
    y = np.zeros((L, HID), dtype=np.float32)
    for c in range(NCORES):
        y += np.asarray(res.results[c]["y"], dtype=np.float32)
    return y.reshape(B, L, HID), res


def kernel(**inputs):
    y, _ = run(trace=False, **inputs)
    return y
